# revision 15
# baseline (speedup 1.0000x reference)
"""Multi-head causal attention (B=2, S=2048, D=1024, H=16, Dh=64) on 8 TRN2
NeuronCores.  TimelineSim per-core exec: ~122us (baseline was 189us).

Sharding: core c = 4*b + g handles batch b (2-way data parallel) and head
group g (4 heads = a 256-column slice of W_q/W_k/W_v and the matching
256-row slice of W_o).  Each core returns a bf16 partial [S, D]; the host
sums the 4 partials per batch and adds b_o.

Per-core pipeline:
  1. QKV projections as 3-term fp8 hi+lo DoubleRow matmuls (0.5 cyc/row):
     x@W ~= x8@W8 + x8@Wr + xr@W8 with unscaled fp8 residuals; f32 psum,
     descaled into bf16 during the PSUM->SBUF copy (q also folds 1/8).
     All DRAM inputs are host-prequantized and laid out partition-major so
     every DMA is a single instruction with >=512B contiguous rows.
  2. Scores transposed per (head, q-chunk, k-tile): S^T = K^T.T Q^T in
     bf16.  Causal masking is a post-exp 0/1 multiply of each diagonal
     128-block on the otherwise-idle Pool (gpsimd) engine (SBUF->SBUF,
     so Pool's no-PSUM restriction doesn't apply).
  3. exp mostly on Act (f32 psum -> bf16 SBUF); some mask-free blocks use
     a Schraudolph bit-trick on DVE (int16(s*128*log2e + b) viewed as
     bf16) to offload the Act engine.
  4. attn@V flipped: stationary = e^T block, moving = [V_h | ones] so the
     softmax denominator accumulates as psum column 64.  Normalization is
     a per-partition reciprocal scale fused into the PSUM->SBUF copy.
  5. ctx^T via paired 128x512 SBUF->SBUF DMA transposes (3D out APs place
     each 128-col block), bf16 out-projection, bf16 y staged and DMA'd.
Emission is software-pipelined: scores run one block ahead of attn@V, j1
projections and out-projections are queued as fine-grained fillers
consumed inside the Act-bound scores loops, and the tail routes copies
through the then-idle Act engine.
"""

import numpy as np
import ml_dtypes
from contextlib import ExitStack

import concourse.bass as bass
import concourse.bacc as bacc
import concourse.tile as tile
import concourse.mybir as mybir
from concourse.bass_utils import run_bass_kernel_spmd

F32 = mybir.dt.float32
BF16 = mybir.dt.bfloat16
FP8 = mybir.dt.float8e4
AF = mybir.ActivationFunctionType
ALU = mybir.AluOpType
DR = mybir.MatmulPerfMode.DoubleRow

NP_FP8 = ml_dtypes.float8_e4m3
NP_BF16 = ml_dtypes.bfloat16

B = 2
S = 2048
D = 1024
DC = 256          # head dims per core (4 heads x 64)
N_CORES = 8
NT = D // 128     # 8 input-dim tiles
ST = S // 128     # 16 sequence tiles
MSK = 20.0        # causal mask slope (per unit of k-q)
CQ = 0.15         # q/k fp8 storage scale (psum sigma 41 -> stored sigma ~6)
ESC = 1.0 / (2.0 * (64.0 * CQ) ** 2 * 8.0)   # exp input scale


def _bank_slices(a, b):
    """Split columns [a, b) at 512-float PSUM bank boundaries."""
    out = []
    while a < b:
        nxt = min(b, (a // 512 + 1) * 512)
        out.append((a, nxt))
        a = nxt
    return out


def _build():
    nc = bacc.Bacc("TRN2", target_bir_lowering=False, debug=False,
                   num_devices=N_CORES)
    xt8 = nc.dram_tensor("xt8", [128, NT * S], FP8, kind="ExternalInput").ap()
    xtr = nc.dram_tensor("xtr", [128, NT * S], FP8, kind="ExternalInput").ap()
    w2 = {}
    for nm in ("q", "k", "v"):
        w2[nm] = nc.dram_tensor(f"w{nm}2", [128, 2 * NT * DC], FP8,
                                kind="ExternalInput").ap()
    wo = nc.dram_tensor("wo", [128, 2 * D], BF16, kind="ExternalInput").ap()
    mk = nc.dram_tensor("mk", [128, 256], BF16, kind="ExternalInput").ap()
    y = nc.dram_tensor("y", [S, D], BF16,
                       kind="ExternalOutput").ap()

    with tile.TileContext(nc) as tc, ExitStack() as stk:
        persist = stk.enter_context(tc.tile_pool(name="persist", bufs=1))
        x8_sb = persist.tile([128, NT, S], FP8)
        xr_sb = persist.tile([128, NT, S], FP8)
        w8_sb = {}
        wr_sb = {}
        w2_sb = {}
        for nm in ("q", "k", "v"):
            w2_sb[nm] = persist.tile([128, 2, NT, DC], FP8, name=f"w2_{nm}")
            w8_sb[nm] = w2_sb[nm][:, 0, :, :]
            wr_sb[nm] = w2_sb[nm][:, 1, :, :]
        wo_sb = persist.tile([128, 2, D], BF16)
        mk_sb = persist.tile([128, 256], BF16)
        mka_sb = mk_sb[:, 0:128]
        mkb_sb = mk_sb[:, 128:256]
        # Q^T/K^T stored fp8 (sigma ~6); scores run as fp8 DoubleRow
        # matmuls with a stride-0 broadcast pair dim (computes 2*K^T.T@Q^T
        # at 0.5 cyc/row; the 2x and the 1/8 softmax scale fold into the
        # exp input scale ESC).
        qt_sb = persist.tile([128, 2, S], FP8)   # Q^T: j-tile of dims x seq
        kt_sb = persist.tile([128, 2, S], FP8)
        v2_sb = persist.tile([128, ST, 4, 65], BF16)  # [k, ktile, head, V|1]
        ctx_sb = persist.tile([128, ST, DC], BF16)    # [q, qtile, dims]
        ctxT_sb = persist.tile([128, 8, 512], BF16)   # paired-T blocks
        rcp_sb = persist.tile([128, 64], F32)         # 1/denom per (h,qt)

        # ---- input DMAs (SP queue); DRAM is pre-arranged in SBUF
        # layout so every transfer is contiguous >=512B rows.  Ordered by
        # first use: hi weights split from residuals so j0 term0 can start
        # after just w8_q + the first x8 chunk ----
        NC2 = NT * DC

        def dma_w(nm, half):
            nc.sync.dma_start(
                out=w2_sb[nm][:, half, :, :],
                in_=w2[nm][:, NC2 * half:NC2 * (half + 1)]
                .rearrange("p (i c) -> p i c", c=DC))

        def dma_x(sb, src, t):
            nc.sync.dma_start(
                out=sb[:, 2 * t:2 * t + 2, :],
                in_=src[:, 2 * S * t:2 * S * (t + 1)]
                .rearrange("p (i s) -> p i s", s=S))

        dma_w("q", 0)
        dma_x(x8_sb, xt8, 0)
        dma_w("k", 0)
        for t in range(1, NT // 2):
            dma_x(x8_sb, xt8, t)
        dma_w("q", 1)
        dma_w("k", 1)
        nc.sync.dma_start(out=mk_sb[:, :], in_=mk[:, :])
        for t in range(NT // 2):
            dma_x(xr_sb, xtr, t)
        nc.sync.dma_start(out=w2_sb["v"][:, :, :, :], in_=w2["v"][:, :])
        nc.sync.dma_start(out=wo_sb[:, :, :],
                          in_=wo.rearrange("p (d c) -> p d c", c=D))

        for h in range(4):
            nc.vector.memset(v2_sb[:, :, h, 64:65], 1.0)

        # PE p-state warmup: the Tensor engine needs ~3us of continuous
        # work to reach full clock, but the first real matmul can't start
        # until the w8_q + x8 DMAs land (~4.5us).  Burn that window with
        # dummy matmuls into the unit-0 psum region (its first real matmul
        # has start=True, which resets the accumulator).
        wu_l = persist.tile([128, 16], BF16)
        wu_r = persist.tile([128, 512], BF16)
        nc.vector.memset(wu_l[:, :], 0.0)
        nc.vector.memset(wu_r[:, :], 0.0)

        # ---- j0 projections in their own psum pool (closes after) ----
        halves = [(h, ab) for h in range(2) for ab in ((0, 512),
                                                       (512, 1024))]
        units0 = ([("q", h, ab, qt_sb) for h, ab in halves] +
                  [("k", h, ab, kt_sb) for h, ab in halves])
        j0ps = {}
        with tc.tile_pool(name="j0p", bufs=1, space="PSUM") as j0p:
            for i in range(4):
                j0ps[i] = j0p.tile([128, 1024], F32, tag=f"u{i}",
                                   name=f"j0ps{i}")
            terms0 = ((w8_sb, x8_sb, 0, False), (wr_sb, x8_sb, 1, False),
                      (w8_sb, xr_sb, 2, True))
            unit_order = sorted(range(8), key=lambda i: (units0[i][1],
                                                         units0[i][0] != "q"))

            for _ in range(10):
                nc.tensor.matmul(j0ps[0][0:16, 0:512], lhsT=wu_l[:, :],
                                 rhs=wu_r[:, :], start=True, stop=True)

            def j0_mm(wsbs, xsbs, u, t, start, stop):
                nm, half, (a, b), dst = units0[u]
                ps = j0ps[u // 2][:, 512 * (u % 2):512 * (u % 2 + 1)]
                nc.tensor.matmul(
                    ps,
                    lhsT=wsbs[nm][:, 2 * t:2 * t + 2, 0:128],
                    rhs=xsbs[:, 2 * t:2 * t + 2,
                             1024 * half + a:1024 * half + b],
                    start=start, stop=stop, perf_mode=DR)

            # t-outer: each x8/xr chunk is consumed by all 8 units as soon
            # as its DMA lands, so j0 tracks the input stream instead of
            # stalling a whole unit on the last chunk.
            for wsbs, xsbs, ti, is_last in terms0:
                for t in range(NT // 2):
                    last = is_last and t == NT // 2 - 1
                    for u in (unit_order if last else range(8)):
                        j0_mm(wsbs, xsbs, u, t,
                              start=(ti == 0 and t == 0), stop=last)
                        if last:
                            nm, half, (a, b), dst = units0[u]
                            scale = CQ
                            dd = dst[:, 0, 1024 * half + a:1024 * half + b]
                            ps = j0ps[u // 2][:, 512 * (u % 2):
                                              512 * (u % 2 + 1)]
                            if u % 2 == 0:
                                nc.vector.tensor_scalar_mul(dd, ps, scale)
                            else:
                                nc.scalar.mul(dd, ps, scale)

        # ---- whole pipeline shares one PSUM pool (tags: big/ap/sy) ----
        with tc.tile_pool(name="ps", bufs=2, space="PSUM") as pspool, \
             tc.tile_pool(name="ep", bufs=2) as ep, \
             tc.tile_pool(name="yb", bufs=3) as yb:

            fillers = []

            def fill_one():
                if fillers:
                    fillers.pop(0)()

            def drain_fillers():
                while fillers:
                    fillers.pop(0)()

            def proj_qk_unit(nm, j, half, ab, dst):
                """One [128,512] slice of a Q^T/K^T tile: 12 DR matmuls."""
                a, b = ab
                ps = pspool.tile([128, 512], F32, tag="sy",
                                 name=f"p{nm}{j}{half}{a}")
                terms = ((w8_sb[nm], x8_sb), (wr_sb[nm], x8_sb),
                         (w8_sb[nm], xr_sb))
                for ti, (wsb, xsb) in enumerate(terms):
                    for t in range(NT // 2):
                        nc.tensor.matmul(
                            ps[:, :],
                            lhsT=wsb[:, 2 * t:2 * t + 2,
                                     128 * j:128 * (j + 1)],
                            rhs=xsb[:, 2 * t:2 * t + 2,
                                    1024 * half + a:1024 * half + b],
                            start=(ti == 0 and t == 0),
                            stop=(ti == 2 and t == NT // 2 - 1),
                            perf_mode=DR)
                nc.vector.tensor_scalar_mul(
                    dst[:, j, 1024 * half + a:1024 * half + b], ps[:, :],
                    CQ)

            def proj_v(st):
                """V tile: psum[128 seq, 256 dv] -> v2_sb[:, st, :, 0:64]."""
                ps = pspool.tile([128, 256], F32, tag="sy", name=f"pv{st}")
                terms = ((x8_sb, w8_sb["v"]), (x8_sb, wr_sb["v"]),
                         (xr_sb, w8_sb["v"]))
                n = 0
                for xsb, wsb in terms:
                    for t in range(NT // 2):
                        nc.tensor.matmul(
                            ps[:, 0:256],
                            lhsT=xsb[:, 2 * t:2 * t + 2,
                                     128 * st:128 * (st + 1)],
                            rhs=wsb[:, 2 * t:2 * t + 2, :],
                            start=(n == 0), stop=(n == 11),
                            perf_mode=DR)
                        n += 1
                nc.vector.tensor_scalar_mul(
                    v2_sb[:, st, :, 0:64],
                    ps[:, :].rearrange("p (h c) -> p h c", c=64),
                    1.0 / 64.0)

            e_tiles = {}

            def scores_exp(h, qc, fill=False, dve_kts=()):
                """S^T + mask into psum, exp -> e_sb[:, kt, :] (bf16)."""
                jh, hb = h // 2, 64 * (h % 2)
                e_sb = ep.tile([128, ST, 1024], BF16, tag="e", name=f"e{h}{qc}")
                e_tiles[(h, qc)] = e_sb
                for kt in range(8 * qc + 8):
                    off = max(0, 128 * kt - 1024 * qc)
                    s_ps = pspool.tile([128, 1024], F32, tag="big",
                                       name=f"s{h}{qc}{kt}")
                    s_base = 0
                    lhsT = kt_sb[hb:hb + 64, jh,
                                 128 * kt:128 * (kt + 1)] \
                        .unsqueeze(1).broadcast_to([64, 2, 128])
                    for a, b in _bank_slices(off, 1024):
                        nc.tensor.matmul(
                            s_ps[:, a - s_base:b - s_base],
                            lhsT=lhsT,
                            rhs=qt_sb[hb:hb + 64, jh,
                                      1024 * qc + a:1024 * qc + b]
                            .unsqueeze(1).broadcast_to([64, 2, b - a]),
                            start=True, stop=True, perf_mode=DR)
                    if kt in dve_kts:
                        # Schraudolph exp on DVE: bf16 bits = s*128*log2(e)
                        # + 128*127 + c  (c tuned for truncating convert).
                        # Writes the full block; cols < off are never read,
                        # and the diagonal block still gets the Pool mask.
                        nc.vector.tensor_scalar(
                            e_sb[:, kt, 0:1024].bitcast(mybir.dt.int16),
                            s_ps[:, 0:1024], 184.6645 * ESC, 16249.0,
                            ALU.mult, ALU.add)
                    else:
                        nc.scalar.activation(e_sb[:, kt, off:1024],
                                             s_ps[:, off - s_base:1024 - s_base],
                                             AF.Exp, scale=ESC)
                    if kt >= 8 * qc:
                        # zero strictly-upper (k > q) of the diagonal
                        # 128-block on the otherwise-idle Pool engine
                        nc.gpsimd.tensor_mul(
                            e_sb[:, kt, off:off + 128],
                            e_sb[:, kt, off:off + 128], mka_sb)
                    if fill:
                        fill_one()

            def attn_v(h, qc, per_qt=None, fill=False):
                """ctx[q, 64]+denom per local q tile; normalize to ctx_sb.

                q tiles run in groups of 4 sharing one psum tile so the
                (expensive, fixed-cost) DVE reciprocal is batched 4-wide."""
                e_sb = e_tiles.pop((h, qc))
                for grp in range(2):
                    a_ps = pspool.tile([128, 4, 65], F32, tag="ap",
                                       name=f"a{h}{qc}{grp}")
                    qts = [8 * qc + 4 * grp + i for i in range(4)]
                    for i, qt in enumerate(qts):
                        qtl = 4 * grp + i
                        for kt in range(qt + 1):
                            nc.tensor.matmul(
                                a_ps[:, i, 0:65],
                                lhsT=e_sb[:, kt, 128 * qtl:128 * (qtl + 1)],
                                rhs=v2_sb[:, kt, h, :],
                                start=(kt == 0), stop=(kt == qt))
                        if fill:
                            fill_one()
                    r4 = rcp_sb[:, 16 * h + qts[0]:16 * h + qts[0] + 4]
                    nc.vector.reciprocal(r4, a_ps[:, :, 64])
                    for i, qt in enumerate(qts):
                        nc.vector.tensor_scalar_mul(
                            ctx_sb[:, qt, 64 * h:64 * (h + 1)],
                            a_ps[:, i, 0:64],
                            rcp_sb[:, 16 * h + qt:16 * h + qt + 1])
                        if per_qt is not None:
                            per_qt(qt)
                        if fill:
                            fill_one()

            def out_proj(qtp, late=False):
                """Paired out-projection for qts (2*qtp, 2*qtp+1): one DMA
                transpose of ctx[256 q, 256 d] into a [128, 512] tile whose
                columns interleave (q-row, qt, d-half); the matmul reads it
                back with stride-4 column APs.  late=True routes copies/DMA
                via the Act queue (idle after exp)."""
                teng = nc.sync
                teng.dma_start_transpose(
                    out=ctxT_sb[:, qtp, :].rearrange("p (m q) -> p m q",
                                                     q=128),
                    in_=ctx_sb[:, 2 * qtp:2 * qtp + 2, :])
                ctp = ctxT_sb[:, qtp, :].rearrange("p (m q) -> p m q", q=128)
                y_sb = yb.tile([128, 2, 1024], BF16, tag="ysb",
                               name=f"ysb{qtp}")
                for j in range(2):
                    qt = 2 * qtp + j
                    for a, b in ((0, 512), (512, 1024)):
                        y_ps = pspool.tile([128, 512], F32, tag="sy",
                                           name=f"y{qt}{a}")
                        for d in range(2):
                            nc.tensor.matmul(
                                y_ps[:, :],
                                lhsT=ctp[:, 2 * j + d, :],
                                rhs=wo_sb[:, d, a:b],
                                start=(d == 0), stop=(d == 1))
                        if j == (0 if late else 1):
                            nc.scalar.copy(y_sb[:, j, a:b], y_ps[:, :])
                        else:
                            nc.vector.tensor_copy(y_sb[:, j, a:b], y_ps[:, :])
                teng.dma_start(
                    out=y[256 * qtp:256 * (qtp + 1), :]
                    .rearrange("(j p) c -> p j c", p=128),
                    in_=y_sb[:, :, :])

            # Emission schedule: j0 projections run directly (DMA-paced
            # startup); everything else enters as fine-grained fillers
            # consumed inside the exp-bound scores loops so the PE never
            # starves.  Scores run one block ahead of attn@V.  The qc=1
            # units (16 exp blocks each) are processed BEFORE the cheap
            # qc=0 units of heads 2,3 so the final pipeline unit has the
            # shortest possible exp drain, and the qc=1 out-projections
            # (transposes + y DMAs) move off the critical tail.
            DK0 = (0, 1)              # Schraudolph blocks, qc=0 units
            DK1 = (0, 1, 2, 3, 4)     # Schraudolph blocks, qc=1 units
            fillers += [lambda st=st: proj_v(st) for st in range(8)]

            scores_exp(0, 0, fill=True, dve_kts=DK0)
            scores_exp(1, 0, fill=True, dve_kts=DK0)
            drain_fillers()        # v[0..7] complete
            attn_v(0, 0)
            scores_exp(0, 1, fill=True, dve_kts=DK1)
            attn_v(1, 0)
            fillers += [lambda st=st: proj_v(st) for st in range(8, ST)]
            fillers += [lambda h=h, ab=ab: proj_qk_unit("q", 1, h, ab, qt_sb)
                        for h, ab in halves]
            fillers += [lambda h=h, ab=ab: proj_qk_unit("k", 1, h, ab, kt_sb)
                        for h, ab in halves]
            scores_exp(1, 1, fill=True, dve_kts=DK1)
            attn_v(0, 1, fill=True)
            drain_fillers()        # v[8..] + j1 q/k complete
            scores_exp(2, 1, dve_kts=DK1)
            attn_v(1, 1)
            scores_exp(3, 1, dve_kts=DK1)
            attn_v(2, 1)
            attn_v(3, 1,
                   per_qt=lambda qt: (fillers.append(
                       lambda q=qt: out_proj(q // 2))
                       if qt % 2 == 1 else None))
            scores_exp(2, 0, fill=True, dve_kts=DK0)   # consumes otp 4..7
            scores_exp(3, 0, fill=True, dve_kts=DK0)
            attn_v(2, 0, fill=True)
            attn_v(3, 0, fill=True,
                   per_qt=lambda qt: (out_proj(qt // 2, late=True)
                                      if qt % 2 == 1 else None))
            drain_fillers()

    nc.compile()
    return nc


_nc = None


def _quant_split(a):
    """fp8 value + unscaled fp8 residual."""
    hi = np.asarray(a, NP_FP8)
    lo = np.asarray(a - hi.astype(np.float32), NP_FP8)
    return hi, lo


def _sbufify(a):
    """[128*NT, C] -> [128, NT*C] partition-major (SBUF tile layout)."""
    n = a.shape[0] // 128
    return np.ascontiguousarray(
        a.reshape(n, 128, -1).transpose(1, 0, 2).reshape(128, -1))


def make_in_maps(x, W_q, W_k, W_v, W_o):
    mka = (np.arange(128)[:, None] <= np.arange(128)[None, :])
    mk = np.concatenate([mka.astype(NP_BF16),
                         np.zeros((128, 128), NP_BF16)], axis=1)
    in_maps = []
    for c in range(N_CORES):
        b, g = c // 4, c % 4
        sl = slice(DC * g, DC * (g + 1))
        x8, xr = _quant_split(np.ascontiguousarray(x[b].T))
        im = {"xt8": _sbufify(x8), "xtr": _sbufify(xr), "mk": mk,
              "wo": _sbufify(np.ascontiguousarray(W_o[sl, :])
                             .astype(NP_BF16))}
        for nm, W in (("q", W_q), ("k", W_k), ("v", W_v)):
            hi, lo = _quant_split(np.ascontiguousarray(W[:, sl]) * 64.0)
            im[f"w{nm}2"] = np.concatenate(
                [_sbufify(hi), _sbufify(lo)], axis=1)
        in_maps.append(im)
    return in_maps


def kernel(x, W_q, W_k, W_v, W_o, b_o):
    global _nc
    x = np.asarray(x, dtype=np.float32)
    W_q = np.asarray(W_q, dtype=np.float32)
    W_k = np.asarray(W_k, dtype=np.float32)
    W_v = np.asarray(W_v, dtype=np.float32)
    W_o = np.asarray(W_o, dtype=np.float32)
    b_o = np.asarray(b_o, dtype=np.float32)

    if _nc is None:
        _nc = _build()

    in_maps = make_in_maps(x, W_q, W_k, W_v, W_o)
    res = run_bass_kernel_spmd(_nc, in_maps, list(range(N_CORES)))
    out = np.empty((B, S, D), dtype=np.float32)
    for b in range(B):
        acc = np.zeros((S, D), dtype=np.float64)
        for g in range(4):
            acc += np.asarray(res.results[4 * b + g]["y"], np.float32)
        acc += b_o
        out[b] = acc.astype(np.float32)
    return out



# revision 16
# speedup vs baseline: 1.1467x; 1.1467x over previous
"""Multi-head causal attention (B=2, S=2048, D=1024, H=16, Dh=64) on 8 TRN2
NeuronCores.  TimelineSim per-core exec: ~122us (baseline was 189us).

Sharding: core c = 4*b + g handles batch b (2-way data parallel) and head
group g (4 heads = a 256-column slice of W_q/W_k/W_v and the matching
256-row slice of W_o).  Each core returns a bf16 partial [S, D]; the host
sums the 4 partials per batch and adds b_o.

Per-core pipeline:
  1. QKV projections as 3-term fp8 hi+lo DoubleRow matmuls (0.5 cyc/row):
     x@W ~= x8@W8 + x8@Wr + xr@W8 with unscaled fp8 residuals; f32 psum,
     descaled into bf16 during the PSUM->SBUF copy (q also folds 1/8).
     All DRAM inputs are host-prequantized and laid out partition-major so
     every DMA is a single instruction with >=512B contiguous rows.
  2. Scores transposed per (head, q-chunk, k-tile): S^T = K^T.T Q^T in
     bf16.  Causal masking is a post-exp 0/1 multiply of each diagonal
     128-block on the otherwise-idle Pool (gpsimd) engine (SBUF->SBUF,
     so Pool's no-PSUM restriction doesn't apply).
  3. exp mostly on Act (f32 psum -> bf16 SBUF); some mask-free blocks use
     a Schraudolph bit-trick on DVE (int16(s*128*log2e + b) viewed as
     bf16) to offload the Act engine.
  4. attn@V flipped: stationary = e^T block, moving = [V_h | ones] so the
     softmax denominator accumulates as psum column 64.  Normalization is
     a per-partition reciprocal scale fused into the PSUM->SBUF copy.
  5. ctx^T via paired 128x512 SBUF->SBUF DMA transposes (3D out APs place
     each 128-col block), bf16 out-projection, bf16 y staged and DMA'd.
Emission is software-pipelined: scores run one block ahead of attn@V, j1
projections and out-projections are queued as fine-grained fillers
consumed inside the Act-bound scores loops, and the tail routes copies
through the then-idle Act engine.
"""

import numpy as np
import ml_dtypes
from contextlib import ExitStack

import concourse.bass as bass
import concourse.bacc as bacc
import concourse.tile as tile
import concourse.mybir as mybir
from concourse.bass_utils import run_bass_kernel_spmd

F32 = mybir.dt.float32
BF16 = mybir.dt.bfloat16
FP8 = mybir.dt.float8e4
AF = mybir.ActivationFunctionType
ALU = mybir.AluOpType
DR = mybir.MatmulPerfMode.DoubleRow

NP_FP8 = ml_dtypes.float8_e4m3
NP_BF16 = ml_dtypes.bfloat16

B = 2
S = 2048
D = 1024
DC = 256          # head dims per core (4 heads x 64)
N_CORES = 8
NT = D // 128     # 8 input-dim tiles
ST = S // 128     # 16 sequence tiles
MSK = 20.0        # causal mask slope (per unit of k-q)
CQ = 0.15         # q/k fp8 storage scale (psum sigma 41 -> stored sigma ~6)
ESC = 1.0 / (2.0 * (64.0 * CQ) ** 2 * 8.0)   # exp input scale


def _bank_slices(a, b):
    """Split columns [a, b) at 512-float PSUM bank boundaries."""
    out = []
    while a < b:
        nxt = min(b, (a // 512 + 1) * 512)
        out.append((a, nxt))
        a = nxt
    return out


def _build():
    nc = bacc.Bacc("TRN2", target_bir_lowering=False, debug=False,
                   num_devices=N_CORES)
    xt8 = nc.dram_tensor("xt8", [128, NT * S], FP8, kind="ExternalInput").ap()
    xtr = nc.dram_tensor("xtr", [128, NT * S], FP8, kind="ExternalInput").ap()
    w2 = {}
    for nm in ("q", "k", "v"):
        w2[nm] = nc.dram_tensor(f"w{nm}2", [128, 2 * NT * DC], FP8,
                                kind="ExternalInput").ap()
    wo = nc.dram_tensor("wo", [128, 2 * D], BF16, kind="ExternalInput").ap()
    mk = nc.dram_tensor("mk", [128, 256], BF16, kind="ExternalInput").ap()
    y = nc.dram_tensor("y", [S, D], BF16,
                       kind="ExternalOutput").ap()

    with tile.TileContext(nc) as tc, ExitStack() as stk:
        persist = stk.enter_context(tc.tile_pool(name="persist", bufs=1))
        x8_sb = persist.tile([128, NT, S], FP8)
        xr_sb = persist.tile([128, NT, S], FP8)
        w8_sb = {}
        wr_sb = {}
        w2_sb = {}
        for nm in ("q", "k", "v"):
            w2_sb[nm] = persist.tile([128, 2, NT, DC], FP8, name=f"w2_{nm}")
            w8_sb[nm] = w2_sb[nm][:, 0, :, :]
            wr_sb[nm] = w2_sb[nm][:, 1, :, :]
        wo_sb = persist.tile([128, 2, D], BF16)
        mk_sb = persist.tile([128, 256], BF16)
        mka_sb = mk_sb[:, 0:128]
        mkb_sb = mk_sb[:, 128:256]
        # Q^T/K^T stored fp8 (sigma ~6); scores run as fp8 DoubleRow
        # matmuls with a stride-0 broadcast pair dim (computes 2*K^T.T@Q^T
        # at 0.5 cyc/row; the 2x and the 1/8 softmax scale fold into the
        # exp input scale ESC).
        qt_sb = persist.tile([128, 2, S], FP8)   # Q^T: j-tile of dims x seq
        kt_sb = persist.tile([128, 2, S], FP8)
        v2_sb = persist.tile([128, ST, 4, 65], BF16)  # [k, ktile, head, V|1]
        ctx_sb = persist.tile([128, ST, DC], BF16)    # [q, qtile, dims]
        ctxT_sb = persist.tile([128, 8, 512], BF16)   # paired-T blocks
        rcp_sb = persist.tile([128, 64], F32)         # 1/denom per (h,qt)

        # ---- input DMAs (SP queue); DRAM is pre-arranged in SBUF
        # layout so every transfer is contiguous >=512B rows.  Ordered by
        # first use: hi weights split from residuals so j0 term0 can start
        # after just w8_q + the first x8 chunk ----
        NC2 = NT * DC

        def dma_w(nm, half):
            nc.sync.dma_start(
                out=w2_sb[nm][:, half, :, :],
                in_=w2[nm][:, NC2 * half:NC2 * (half + 1)]
                .rearrange("p (i c) -> p i c", c=DC))

        def dma_x(sb, src, t):
            nc.sync.dma_start(
                out=sb[:, 2 * t:2 * t + 2, :],
                in_=src[:, 2 * S * t:2 * S * (t + 1)]
                .rearrange("p (i s) -> p i s", s=S))

        dma_w("q", 0)
        dma_x(x8_sb, xt8, 0)
        dma_w("k", 0)
        for t in range(1, NT // 2):
            dma_x(x8_sb, xt8, t)
        dma_w("q", 1)
        dma_w("k", 1)
        nc.sync.dma_start(out=mk_sb[:, :], in_=mk[:, :])
        for t in range(NT // 2):
            dma_x(xr_sb, xtr, t)
        nc.sync.dma_start(out=w2_sb["v"][:, :, :, :], in_=w2["v"][:, :])
        nc.sync.dma_start(out=wo_sb[:, :, :],
                          in_=wo.rearrange("p (d c) -> p d c", c=D))

        for h in range(4):
            nc.vector.memset(v2_sb[:, :, h, 64:65], 1.0)

        # PE p-state warmup: the Tensor engine needs ~3us of continuous
        # work to reach full clock, but the first real matmul can't start
        # until the w8_q + x8 DMAs land (~4.5us).  Burn that window with
        # dummy matmuls into the unit-0 psum region (its first real matmul
        # has start=True, which resets the accumulator).
        wu_l = persist.tile([128, 16], BF16)
        wu_r = persist.tile([128, 512], BF16)
        nc.vector.memset(wu_l[:, :], 0.0)
        nc.vector.memset(wu_r[:, :], 0.0)

        # ---- j0 projections in their own psum pool (closes after) ----
        halves = [(h, ab) for h in range(2) for ab in ((0, 512),
                                                       (512, 1024))]
        units0 = ([("q", h, ab, qt_sb) for h, ab in halves] +
                  [("k", h, ab, kt_sb) for h, ab in halves])
        j0ps = {}
        with tc.tile_pool(name="j0p", bufs=1, space="PSUM") as j0p:
            for i in range(4):
                j0ps[i] = j0p.tile([128, 1024], F32, tag=f"u{i}",
                                   name=f"j0ps{i}")
            terms0 = ((w8_sb, x8_sb, 0, False), (wr_sb, x8_sb, 1, False),
                      (w8_sb, xr_sb, 2, True))
            unit_order = sorted(range(8), key=lambda i: (units0[i][1],
                                                         units0[i][0] != "q"))

            for _ in range(10):
                nc.tensor.matmul(j0ps[0][0:16, 0:512], lhsT=wu_l[:, :],
                                 rhs=wu_r[:, :], start=True, stop=True)

            def j0_mm(wsbs, xsbs, u, t, start, stop):
                nm, half, (a, b), dst = units0[u]
                ps = j0ps[u // 2][:, 512 * (u % 2):512 * (u % 2 + 1)]
                nc.tensor.matmul(
                    ps,
                    lhsT=wsbs[nm][:, 2 * t:2 * t + 2, 0:128],
                    rhs=xsbs[:, 2 * t:2 * t + 2,
                             1024 * half + a:1024 * half + b],
                    start=start, stop=stop, perf_mode=DR)

            # t-outer: each x8/xr chunk is consumed by all 8 units as soon
            # as its DMA lands, so j0 tracks the input stream instead of
            # stalling a whole unit on the last chunk.
            for wsbs, xsbs, ti, is_last in terms0:
                for t in range(NT // 2):
                    last = is_last and t == NT // 2 - 1
                    for u in (unit_order if last else range(8)):
                        j0_mm(wsbs, xsbs, u, t,
                              start=(ti == 0 and t == 0), stop=last)
                        if last:
                            nm, half, (a, b), dst = units0[u]
                            scale = CQ
                            dd = dst[:, 0, 1024 * half + a:1024 * half + b]
                            ps = j0ps[u // 2][:, 512 * (u % 2):
                                              512 * (u % 2 + 1)]
                            if u % 2 == 0:
                                nc.vector.tensor_scalar_mul(dd, ps, scale)
                            else:
                                nc.scalar.mul(dd, ps, scale)

        # ---- whole pipeline shares one PSUM pool (tags: big/ap/sy) ----
        with tc.tile_pool(name="ps", bufs=2, space="PSUM") as pspool, \
             tc.tile_pool(name="ep", bufs=2) as ep, \
             tc.tile_pool(name="yb", bufs=3) as yb:

            fillers = []

            def fill_one():
                if fillers:
                    fillers.pop(0)()

            def drain_fillers():
                while fillers:
                    fillers.pop(0)()

            def proj_qk_unit(nm, j, half, ab, dst):
                """One [128,512] slice of a Q^T/K^T tile: 12 DR matmuls."""
                a, b = ab
                ps = pspool.tile([128, 512], F32, tag="sy",
                                 name=f"p{nm}{j}{half}{a}")
                terms = ((w8_sb[nm], x8_sb), (wr_sb[nm], x8_sb),
                         (w8_sb[nm], xr_sb))
                for ti, (wsb, xsb) in enumerate(terms):
                    for t in range(NT // 2):
                        nc.tensor.matmul(
                            ps[:, :],
                            lhsT=wsb[:, 2 * t:2 * t + 2,
                                     128 * j:128 * (j + 1)],
                            rhs=xsb[:, 2 * t:2 * t + 2,
                                    1024 * half + a:1024 * half + b],
                            start=(ti == 0 and t == 0),
                            stop=(ti == 2 and t == NT // 2 - 1),
                            perf_mode=DR)
                nc.vector.tensor_scalar_mul(
                    dst[:, j, 1024 * half + a:1024 * half + b], ps[:, :],
                    CQ)

            def proj_v(st):
                """V tile: psum[128 seq, 256 dv] -> v2_sb[:, st, :, 0:64]."""
                ps = pspool.tile([128, 256], F32, tag="sy", name=f"pv{st}")
                terms = ((x8_sb, w8_sb["v"]), (x8_sb, wr_sb["v"]),
                         (xr_sb, w8_sb["v"]))
                n = 0
                for xsb, wsb in terms:
                    for t in range(NT // 2):
                        nc.tensor.matmul(
                            ps[:, 0:256],
                            lhsT=xsb[:, 2 * t:2 * t + 2,
                                     128 * st:128 * (st + 1)],
                            rhs=wsb[:, 2 * t:2 * t + 2, :],
                            start=(n == 0), stop=(n == 11),
                            perf_mode=DR)
                        n += 1
                nc.vector.tensor_scalar_mul(
                    v2_sb[:, st, :, 0:64],
                    ps[:, :].rearrange("p (h c) -> p h c", c=64),
                    1.0 / 64.0)

            e_tiles = {}

            def scores_exp(h, qc, fill=False, dve_kts=()):
                """S^T + mask into psum, exp -> e_sb[:, kt, :] (bf16)."""
                jh, hb = h // 2, 64 * (h % 2)
                e_sb = ep.tile([128, ST, 1024], BF16, tag="e", name=f"e{h}{qc}")
                e_tiles[(h, qc)] = e_sb
                for kt in range(8 * qc + 8):
                    off = max(0, 128 * kt - 1024 * qc)
                    s_ps = pspool.tile([128, 1024], F32, tag="big",
                                       name=f"s{h}{qc}{kt}")
                    s_base = 0
                    lhsT = kt_sb[hb:hb + 64, jh,
                                 128 * kt:128 * (kt + 1)] \
                        .unsqueeze(1).broadcast_to([64, 2, 128])
                    for a, b in _bank_slices(off, 1024):
                        nc.tensor.matmul(
                            s_ps[:, a - s_base:b - s_base],
                            lhsT=lhsT,
                            rhs=qt_sb[hb:hb + 64, jh,
                                      1024 * qc + a:1024 * qc + b]
                            .unsqueeze(1).broadcast_to([64, 2, b - a]),
                            start=True, stop=True, perf_mode=DR)
                    if kt in dve_kts:
                        # Schraudolph exp on DVE: bf16 bits = s*128*log2(e)
                        # + 128*127 + c  (c tuned for truncating convert).
                        # Writes the full block; cols < off are never read,
                        # and the diagonal block still gets the Pool mask.
                        nc.vector.tensor_scalar(
                            e_sb[:, kt, 0:1024].bitcast(mybir.dt.int16),
                            s_ps[:, 0:1024], 184.6645 * ESC, 16249.0,
                            ALU.mult, ALU.add)
                    else:
                        nc.scalar.activation(e_sb[:, kt, off:1024],
                                             s_ps[:, off - s_base:1024 - s_base],
                                             AF.Exp, scale=ESC)
                    if kt >= 8 * qc:
                        # zero strictly-upper (k > q) of the diagonal
                        # 128-block on the otherwise-idle Pool engine
                        nc.gpsimd.tensor_mul(
                            e_sb[:, kt, off:off + 128],
                            e_sb[:, kt, off:off + 128], mka_sb)
                    if fill:
                        fill_one()

            def attn_v(h, qc, per_qt=None, fill=False):
                """ctx[q, 64]+denom per local q tile; normalize to ctx_sb.

                q tiles run in groups of 4 sharing one psum tile so the
                (expensive, fixed-cost) DVE reciprocal is batched 4-wide."""
                e_sb = e_tiles.pop((h, qc))
                for grp in range(2):
                    a_ps = pspool.tile([128, 4, 65], F32, tag="ap",
                                       name=f"a{h}{qc}{grp}")
                    qts = [8 * qc + 4 * grp + i for i in range(4)]
                    for i, qt in enumerate(qts):
                        qtl = 4 * grp + i
                        for kt in range(qt + 1):
                            nc.tensor.matmul(
                                a_ps[:, i, 0:65],
                                lhsT=e_sb[:, kt, 128 * qtl:128 * (qtl + 1)],
                                rhs=v2_sb[:, kt, h, :],
                                start=(kt == 0), stop=(kt == qt))
                        if fill:
                            fill_one()
                    r4 = rcp_sb[:, 16 * h + qts[0]:16 * h + qts[0] + 4]
                    nc.vector.reciprocal(r4, a_ps[:, :, 64])
                    for i, qt in enumerate(qts):
                        nc.vector.tensor_scalar_mul(
                            ctx_sb[:, qt, 64 * h:64 * (h + 1)],
                            a_ps[:, i, 0:64],
                            rcp_sb[:, 16 * h + qt:16 * h + qt + 1])
                        if per_qt is not None:
                            per_qt(qt)
                        if fill:
                            fill_one()

            def out_proj(qtp, late=False):
                """Paired out-projection for qts (2*qtp, 2*qtp+1): one DMA
                transpose of ctx[256 q, 256 d] into a [128, 512] tile whose
                columns interleave (q-row, qt, d-half); the matmul reads it
                back with stride-4 column APs.  late=True routes copies/DMA
                via the Act queue (idle after exp)."""
                teng = nc.sync
                teng.dma_start_transpose(
                    out=ctxT_sb[:, qtp, :].rearrange("p (m q) -> p m q",
                                                     q=128),
                    in_=ctx_sb[:, 2 * qtp:2 * qtp + 2, :])
                ctp = ctxT_sb[:, qtp, :].rearrange("p (m q) -> p m q", q=128)
                y_sb = yb.tile([128, 2, 1024], BF16, tag="ysb",
                               name=f"ysb{qtp}")
                for j in range(2):
                    qt = 2 * qtp + j
                    for a, b in ((0, 512), (512, 1024)):
                        y_ps = pspool.tile([128, 512], F32, tag="sy",
                                           name=f"y{qt}{a}")
                        for d in range(2):
                            nc.tensor.matmul(
                                y_ps[:, :],
                                lhsT=ctp[:, 2 * j + d, :],
                                rhs=wo_sb[:, d, a:b],
                                start=(d == 0), stop=(d == 1))
                        if j == (0 if late else 1):
                            nc.scalar.copy(y_sb[:, j, a:b], y_ps[:, :])
                        else:
                            nc.vector.tensor_copy(y_sb[:, j, a:b], y_ps[:, :])
                teng.dma_start(
                    out=y[256 * qtp:256 * (qtp + 1), :]
                    .rearrange("(j p) c -> p j c", p=128),
                    in_=y_sb[:, :, :])

            # Emission schedule: j0 projections run directly (DMA-paced
            # startup); everything else enters as fine-grained fillers
            # consumed inside the exp-bound scores loops so the PE never
            # starves.  Scores run one block ahead of attn@V.  The qc=1
            # units (16 exp blocks each) are processed BEFORE the cheap
            # qc=0 units of heads 2,3 so the final pipeline unit has the
            # shortest possible exp drain, and the qc=1 out-projections
            # (transposes + y DMAs) move off the critical tail.
            DK0 = ()                  # Schraudolph blocks, qc=0 units
            DK1 = (0, 2, 4, 6)        # Schraudolph blocks, qc=1 units
            fillers += [lambda st=st: proj_v(st) for st in range(8)]

            scores_exp(0, 0, fill=True, dve_kts=DK0)
            scores_exp(1, 0, fill=True, dve_kts=DK0)
            drain_fillers()        # v[0..7] complete
            attn_v(0, 0)
            scores_exp(0, 1, fill=True, dve_kts=DK1)
            attn_v(1, 0)
            fillers += [lambda st=st: proj_v(st) for st in range(8, ST)]
            fillers += [lambda h=h, ab=ab: proj_qk_unit("q", 1, h, ab, qt_sb)
                        for h, ab in halves]
            fillers += [lambda h=h, ab=ab: proj_qk_unit("k", 1, h, ab, kt_sb)
                        for h, ab in halves]
            scores_exp(1, 1, fill=True, dve_kts=DK1)
            attn_v(0, 1, fill=True)
            drain_fillers()        # v[8..] + j1 q/k complete
            scores_exp(2, 0, dve_kts=DK0)
            attn_v(1, 1)
            scores_exp(3, 0, dve_kts=DK0)
            attn_v(2, 0)
            scores_exp(2, 1, dve_kts=DK1)
            attn_v(3, 0,
                   per_qt=lambda qt: (fillers.append(
                       lambda q=qt: out_proj(q // 2))
                       if qt % 2 == 1 else None))
            scores_exp(3, 1, fill=True, dve_kts=DK1)   # consumes otp 0..7
            attn_v(2, 1, fill=True)
            attn_v(3, 1, fill=True,
                   per_qt=lambda qt: (out_proj(qt // 2, late=True)
                                      if qt % 2 == 1 else None))
            drain_fillers()

    nc.compile()
    return nc


_nc = None


def _quant_split(a):
    """fp8 value + unscaled fp8 residual."""
    hi = np.asarray(a, NP_FP8)
    lo = np.asarray(a - hi.astype(np.float32), NP_FP8)
    return hi, lo


def _sbufify(a):
    """[128*NT, C] -> [128, NT*C] partition-major (SBUF tile layout)."""
    n = a.shape[0] // 128
    return np.ascontiguousarray(
        a.reshape(n, 128, -1).transpose(1, 0, 2).reshape(128, -1))


def make_in_maps(x, W_q, W_k, W_v, W_o):
    mka = (np.arange(128)[:, None] <= np.arange(128)[None, :])
    mk = np.concatenate([mka.astype(NP_BF16),
                         np.zeros((128, 128), NP_BF16)], axis=1)
    in_maps = []
    for c in range(N_CORES):
        b, g = c // 4, c % 4
        sl = slice(DC * g, DC * (g + 1))
        x8, xr = _quant_split(np.ascontiguousarray(x[b].T))
        im = {"xt8": _sbufify(x8), "xtr": _sbufify(xr), "mk": mk,
              "wo": _sbufify(np.ascontiguousarray(W_o[sl, :])
                             .astype(NP_BF16))}
        for nm, W in (("q", W_q), ("k", W_k), ("v", W_v)):
            hi, lo = _quant_split(np.ascontiguousarray(W[:, sl]) * 64.0)
            im[f"w{nm}2"] = np.concatenate(
                [_sbufify(hi), _sbufify(lo)], axis=1)
        in_maps.append(im)
    return in_maps


def kernel(x, W_q, W_k, W_v, W_o, b_o):
    global _nc
    x = np.asarray(x, dtype=np.float32)
    W_q = np.asarray(W_q, dtype=np.float32)
    W_k = np.asarray(W_k, dtype=np.float32)
    W_v = np.asarray(W_v, dtype=np.float32)
    W_o = np.asarray(W_o, dtype=np.float32)
    b_o = np.asarray(b_o, dtype=np.float32)

    if _nc is None:
        _nc = _build()

    in_maps = make_in_maps(x, W_q, W_k, W_v, W_o)
    res = run_bass_kernel_spmd(_nc, in_maps, list(range(N_CORES)))
    out = np.empty((B, S, D), dtype=np.float32)
    for b in range(B):
        acc = np.zeros((S, D), dtype=np.float64)
        for g in range(4):
            acc += np.asarray(res.results[4 * b + g]["y"], np.float32)
        acc += b_o
        out[b] = acc.astype(np.float32)
    return out

